# revision 26
# baseline (speedup 1.0000x reference)
"""Biased axial attention (RoseTTAFold-style) on 8 TRN2 NeuronCores.

nn_BiasedAxialAttention: O=1, L=384, d_pair=d_bias=128, H=4, DH=32.

  p    = LN(pair^T);  bsrc = LN(bias^T)            (LN over d per position)
  q,k,v,gate projections of p; b = bsrc @ Wb^T
  attn[i,j,h] = sum_{n,k} q[n,i,h,k] k[n,j,h,k] + b[i,j,h]
  A = softmax_j(attn);  out[n,i,:] = (gate * einsum(A, v)) @ Wo^T + bo
  result[i,n,:] = out[n,i,:]

Sharding: the non-attended axial dim n, 48 rows/core. Each core computes
partial logits for its n-slice; two on-chip AllReduces (heads 01 / 23) sum
them. The bias term B (each core computes its i-slice) is shared via two
AllGathers. bo is added on the host during the gather.

v2 pipeline highlights (vs the v1 baseline):
  - LN: batched bn_stats ([128,4,128] per op), fused stats arithmetic on
    GpSimd, DVE-4x/ACT normalize, and the feature transpose via the DMA
    XBAR (dma_start transpose=True) straight into X^T -- no PE transpose,
    no PSUM eviction.
  - Q/K projections packed two-per-PSUM-bank-pair, wide evictions, and the
    (h,k)->(k,nn) partition restack batched as one DMA per (head, 2 groups).
  - logits h-split into two AllReduces so softmax/einsum pipeline by
    head-pair; B projection runs j-major so its evictions are [128,16]
    wide instead of [4,384].
  - softmax: one batched max-reduce per head-pair, exp with fused
    accumulation, A^T via DMA XBAR transpose.
  - einsum: full-width [128,2,384] PSUM evictions + DVE-4x partition
    shifts into the gated layout; gate multiply on GpSimd; output
    projection PSUM-batched 4 row-blocks per bank.
"""
import sys

if "/opt/trn_rl_repo" not in sys.path:
    sys.path.insert(0, "/opt/trn_rl_repo")

import numpy as np
from contextlib import ExitStack

import concourse.bass as bass
import concourse.bacc as bacc
import concourse.mybir as mybir
import concourse.tile as tile
from concourse.bass_utils import run_bass_kernel_spmd
from concourse.masks import make_identity

F32 = mybir.dt.float32
BF16 = mybir.dt.bfloat16
AF = mybir.ActivationFunctionType
ALU = mybir.AluOpType
AX = mybir.AxisListType

O, L, DP, H, DH = 1, 384, 128, 4, 32
HD = H * DH
NCORES = 8
NS = L // NCORES            # 48 n's per core
R = NS * L                  # 18432 rows per core
NBLK = R // 128             # 144 row-blocks
NG = NS // 4                # 12 logit contraction groups (4 n's each)
IB = L // 128               # 3 blocks of 128 along i/j
SCALING = 1.0 / np.sqrt(DH)
EPS = 1e-5

CH = 12                     # LN chunk: 12 row-blocks = 1536 rows = one group
NCHUNK = NBLK // CH         # 12 chunks per tensor


def build_tile_kernel(ctx: ExitStack, tc: tile.TileContext, outs, ins):
    nc = tc.nc
    pairc = ins["pairc"].rearrange("(b p) d -> p b d", p=128)   # [128,144,128]
    biasc = ins["biasc"].rearrange("(b p) d -> p b d", p=128)
    outc = outs["outc"].rearrange("(b p) d -> p b d", p=128)

    const = ctx.enter_context(tc.tile_pool(name="const", bufs=1))
    big = ctx.enter_context(tc.tile_pool(name="big", bufs=1))
    lnload = ctx.enter_context(tc.tile_pool(name="lnload", bufs=3))
    stream = ctx.enter_context(tc.tile_pool(name="stream", bufs=2))
    bstream = ctx.enter_context(tc.tile_pool(name="bstream", bufs=3))
    evqp = ctx.enter_context(tc.tile_pool(name="evqp", bufs=2))
    stgp = ctx.enter_context(tc.tile_pool(name="stgp", bufs=3))
    baddp = ctx.enter_context(tc.tile_pool(name="baddp", bufs=2))
    outstg = ctx.enter_context(tc.tile_pool(name="outstg", bufs=2))
    ps_acc = ctx.enter_context(tc.tile_pool(name="psacc", bufs=2, space="PSUM"))
    ps_tr = ctx.enter_context(tc.tile_pool(name="pstr", bufs=2, space="PSUM"))
    ps_pj = ctx.enter_context(tc.tile_pool(name="pspj", bufs=1, space="PSUM"))
    dram = ctx.enter_context(tc.tile_pool(name="dram", bufs=1, space="DRAM"))

    # ---------------- stage 0: constants / weights -------------------------
    ident = const.tile([128, 128], BF16)
    make_identity(nc, ident)
    epst = const.tile([128, 1], F32)
    nc.vector.memset(epst[:], EPS)

    vecs = const.tile([128, 4], F32)          # cols: gp, gb, bg, bo(unused)
    nc.sync.dma_start(vecs[:], ins["vecs"][:])

    wf = {}
    for nm in ("wqt", "wkt", "wvt", "wgt", "wot"):
        t = const.tile([128, 128], F32, name=f"{nm}_f")
        nc.sync.dma_start(t[:], ins[nm][:])
        wf[nm] = t
    wbt_f = const.tile([128, 4], F32)
    nc.sync.dma_start(wbt_f[:], ins["wbt"][:])

    # fold LN gain + scale constants into bf16 lhsT weights (per-partition d)
    wb = {}
    for nm, extra in (("wqt", SCALING), ("wkt", 1.0 / L), ("wvt", 1.0),
                      ("wgt", 1.0)):
        gs = const.tile([128, 1], F32, name=f"{nm}_gs")
        nc.vector.tensor_scalar_mul(gs[:], vecs[:, 0:1], extra)
        t = const.tile([128, 128], BF16, name=f"{nm}_b")
        nc.vector.tensor_scalar_mul(t[:], wf[nm][:], gs[:, 0:1])
        wb[nm] = t
    wot_b = const.tile([128, 128], BF16)
    nc.vector.tensor_copy(wot_b[:], wf["wot"][:])
    wbt_b = const.tile([128, 4], BF16)
    nc.vector.tensor_scalar_mul(wbt_b[:], wbt_f[:], vecs[:, 1:2])

    # ---------------- persistent SBUF / DRAM tensors -----------------------
    xt = big.tile([128, NS, L], BF16, tag="xt")        # X^T [d,(n,i)]
    qs = big.tile([128, H, NG, L], BF16, tag="qs")     # [(k,nn), h, g, i]
    ks = big.tile([128, H, NG, L], BF16, tag="ks")
    zsb = big.tile([128, H, IB, L], BF16, tag="z")     # logits [i%128,h,ib,j]
    asb = big.tile([128, H, IB, L], BF16, tag="a")     # softmax(A)
    at = big.tile([128, H, IB, L], BF16, tag="at")     # A^T [j%128, h, jb, i]

    zin, zout, bgin, bgout = [], [], [], []
    for hp in range(2):
        zin.append(dram.tile([128, 2 * IB * L], BF16, name=f"zin{hp}"))
        zout.append(dram.tile([128, 2 * IB * L], BF16, addr_space="Shared",
                              name=f"zout{hp}"))
        bgin.append(dram.tile([NS, 2, IB, 128], BF16, name=f"bgin{hp}"))
        bgout.append(dram.tile([L, 2, IB, 128], BF16, addr_space="Shared",
                               name=f"bgout{hp}"))

    # ---------------- LN chunk: stats + normalize + XBAR transpose ---------
    # bn_stats is batched 4 blocks/op; the stats arithmetic is fused into 6
    # GpSimd ops on [128,12] tiles; r = rsqrt(var+eps) in one ACT op.
    # The feature-major transpose goes through the DMA XBAR straight into
    # the destination -- no PE transpose, no PSUM round-trip.
    def ln_chunk(src_dram, ch, dst_ap):
        rm = lnload.tile([128, CH, 128], BF16, tag="lnrm")
        nc.gpsimd.dma_start(rm[:], src_dram[:, ch * CH:(ch + 1) * CH, :])
        st = stream.tile([128, CH, 6], F32, tag="lnst")
        for b in range(CH):
            nc.vector.bn_stats(st[:, b, :], rm[:, b, :])
        dm = stream.tile([128, CH], F32, tag="lndm")
        v0 = stream.tile([128, CH], F32, tag="lnv0")
        r = stream.tile([128, CH], F32, tag="lnr")
        nmr = stream.tile([128, CH], F32, tag="lnnmr")
        v = nc.vector
        # var = (cv_e+cv_o)/128 + ((m_e-m_o)/2)^2 ; mean = (m_e+m_o)/2
        # single-engine arithmetic chain: fewer cross-engine hops per chunk
        v.tensor_tensor(dm[:], st[:, :, 1], st[:, :, 4], ALU.subtract)
        v.scalar_tensor_tensor(dm[:], dm[:], 0.25, dm[:], ALU.mult, ALU.mult)
        v.tensor_tensor(v0[:], st[:, :, 2], st[:, :, 5], ALU.add)
        v.scalar_tensor_tensor(v0[:], v0[:], 1.0 / 128, dm[:], ALU.mult,
                               ALU.add)
        nc.scalar.activation(r[:], v0[:], AF.Sqrt, bias=epst[:, 0:1])
        v.reciprocal(r[:], r[:])
        v.tensor_tensor(nmr[:], st[:, :, 1], st[:, :, 4], ALU.add)
        v.scalar_tensor_tensor(nmr[:], nmr[:], -0.5, r[:], ALU.mult,
                               ALU.mult)
        norm = stream.tile([128, CH, 128], BF16, tag="lnnorm")
        for b in range(CH):
            if b % 3 == 2:
                nc.scalar.activation(norm[:, b, :], rm[:, b, :], AF.Identity,
                                     bias=nmr[:, b:b + 1], scale=r[:, b:b + 1])
            else:
                nc.vector.tensor_scalar(norm[:, b, :], rm[:, b, :],
                                        r[:, b:b + 1], nmr[:, b:b + 1],
                                        ALU.mult, ALU.add)
        # PE transpose in 6-block halves, one batched 2x eviction per half
        for half in range(2):
            pt = ps_tr.tile([128, 6, 128], BF16, tag="tr")
            for b in range(6):
                nc.tensor.transpose(pt[:, b, :], norm[:, 6 * half + b, :],
                                    ident[:])
            if half % 2 == 0:
                nc.vector.tensor_copy(dst_ap[:, 6 * half:6 * half + 6, :],
                                      pt[:])
            else:
                nc.scalar.copy(dst_ap[:, 6 * half:6 * half + 6, :], pt[:])

    # ---------------- stage 1+3: pair LN + Q/K projections -----------------
    # qs/ks partition order within a group is p = 4k+nn; one restack DMA per
    # (head, 2 groups) turns evq [32k, gp, nn, i] into qs [(k nn), gp, i].
    # B projection runs j-major: out[j%128, il, h] tiles make the evictions
    # [128,16] instead of [4,384]; a tiny PE transpose flips each chunk to
    # row form [(h jb il), j%128] so the bgin DMA writes 256B runs.
    def bias_chunk(ch):
        btc = bstream.tile([128, 4, L], BF16, tag="btc")
        ln_chunk(biasc, ch,
                 btc[:].rearrange("p n (b q) -> p (n b) q", q=128))
        btj = bstream.tile([128, H, IB, 4], BF16, tag="btj")
        for jb in range(IB):
            bp = ps_pj.tile([128, 4, H], F32, tag="pj")
            for il in range(4):
                nc.tensor.matmul(bp[:, il, :],
                                 btc[:, il, jb * 128:(jb + 1) * 128],
                                 wbt_b[:], start=True, stop=True)
            if jb % 2 == 0:
                nc.vector.tensor_copy(btj[:, :, jb, :],
                                      bp[:].transpose([0, 2, 1]))
            else:
                nc.scalar.copy(btj[:, :, jb, :], bp[:].transpose([0, 2, 1]))
        pt = ps_pj.tile([48, 128], BF16, tag="pjt")
        nc.tensor.transpose(pt[:],
                            btj[:].rearrange("p h b il -> p (h b il)"),
                            ident[:])
        bsg = bstream.tile([48, 128], BF16, tag="bsg")
        nc.vector.tensor_copy(bsg[:], pt[:])
        for hp in range(2):
            dst = bgin[hp][4 * ch:4 * ch + 4, :, :, :]
            nc.gpsimd.dma_start(dst.transpose([1, 2, 0, 3]),
                                bsg[24 * hp:24 * hp + 24, :])

    for g in range(NG):
        dst = xt[:, 4 * g:4 * g + 4, :].rearrange("p n (b q) -> p (n b) q",
                                                  q=128)
        ln_chunk(pairc, g, dst)
        evq = {}
        for wname in ("wqt", "wkt"):
            evq[wname] = evqp.tile([128, 4, L], BF16, tag=f"evq_{wname}",
                                   name=f"evq_{wname}_{g}")
        for wi, wname in enumerate(("wqt", "wkt")):
            for half in range(2):
                pp = ps_acc.tile([128, 2, 512], F32, tag="acc")
                for s in range(2):
                    nn = 2 * half + s
                    nc.tensor.matmul(pp[:, s, 0:L], wb[wname][:],
                                     xt[:, 4 * g + nn, :], start=True,
                                     stop=True)
                dst = evq[wname][:, 2 * half:2 * half + 2, :]
                if (wi + half) % 2 == 0:
                    nc.vector.tensor_copy(dst, pp[:, :, 0:L])
                else:
                    nc.scalar.copy(dst, pp[:, :, 0:L])
        for h in range(H):
            for wname, dstb in (("wqt", qs), ("wkt", ks)):
                eng = nc.sync if h % 2 == 0 else nc.scalar
                eng.dma_start(dstb[:, h, g, :],
                              evq[wname][32 * h:32 * h + 32, :, :])
        bias_chunk(g)

    # AllGathers issued before the AllReduces so the ring starts as soon as
    # the bias side lands; softmax h01 then only waits on AG01+AR1.
    for hp in range(2):
        nc.gpsimd.collective_compute(
            "AllGather", ALU.bypass, replica_groups=[list(range(NCORES))],
            ins=[bgin[hp][:].opt()], outs=[bgout[hp][:].opt()])

    # ---------------- stage 4: logits + h-split AllReduce ------------------
    # bf16 AllReduce carries only the tiny q.k partial sums; the dominant
    # bias term is added post-AR from the AllGather.
    for hp in range(2):
        for h in (2 * hp, 2 * hp + 1):
            for ib in range(IB):
                lp = ps_tr.tile([128, 512], F32, tag="tr")
                for gg in range(NG):
                    nc.tensor.matmul(lp[:, 0:L],
                                     qs[:, h, gg, ib * 128:(ib + 1) * 128],
                                     ks[:, h, gg, :], start=(gg == 0),
                                     stop=(gg == NG - 1))
                if (h + ib) % 2 == 0:
                    nc.vector.tensor_copy(zsb[:, h, ib, :], lp[:, 0:L])
                else:
                    nc.scalar.copy(zsb[:, h, ib, :], lp[:, 0:L])
        nc.sync.dma_start(
            zin[hp][:],
            zsb[:, 2 * hp:2 * hp + 2, :, :].rearrange("p h b j -> p (h b j)"))
        nc.gpsimd.collective_compute(
            "AllReduce", ALU.add, replica_groups=[list(range(NCORES))],
            ins=[zin[hp][:].opt()], outs=[zout[hp][:].opt()])

    # ---------------- stage 6: G + V projections (overlap AR) --------------
    gsb = big.tile([128, NS, L], BF16, tag="ks")       # reuses ks slot
    for np_ in range(NS // 2):
        gp_ = ps_acc.tile([128, 2, 512], F32, tag="acc")
        for s in range(2):
            nc.tensor.matmul(gp_[:, s, 0:L], wb["wgt"][:],
                             xt[:, 2 * np_ + s, :], start=True, stop=True)
        nc.scalar.activation(gsb[:, 2 * np_:2 * np_ + 2, :], gp_[:, :, 0:L],
                             AF.Sigmoid, bias=vecs[:, 2:3])

    # vt layout: [j%128, jb, h, g, nn, d] so the einsum stationary slice is
    # one contiguous 128-wide free dim
    vt = big.tile([128, IB, H, NG, 4, DH], BF16, tag="qs")   # reuses qs slot
    for g in range(NG):
        for jb in range(IB):
            vp = ps_tr.tile([128, 4, 128], F32, tag="tr")
            for s in range(4):
                nc.tensor.matmul(vp[:, s, :],
                                 xt[:, 4 * g + s, jb * 128:(jb + 1) * 128],
                                 wb["wvt"][:], start=True, stop=True)
            vdst = vt[:, jb, :, g, :, :]
            vsrc = vp[:].rearrange("p s (h d) -> p h s d", h=H)
            if (g + jb) % 2 == 0:
                nc.vector.tensor_copy(vdst, vsrc)
            else:
                nc.scalar.copy(vdst, vsrc)

    # ---------------- stage 5..10: per head-pair tail ----------------------
    gated = big.tile([128, NS, L], BF16, tag="xt")     # reuses xt slot
    gated_flat = gated[:].rearrange("p n l -> p (n l)")
    gated_r = gated[:].rearrange("p (a b c) l -> p a b c l", b=2, c=4)
    sums = stream.tile([128, 2, 2, IB], F32, tag="smsum")
    nmx = stream.tile([128, 2, 2, IB], F32, tag="smmax")
    rec = stream.tile([128, 2, 2, IB], F32, tag="smrec")

    for hp in range(2):
        zslc = zsb[:, 2 * hp:2 * hp + 2, :, :]
        zflat = zslc.rearrange("p h b j -> p (h b j)")
        nc.gpsimd.dma_start(zflat, zout[hp][:])
        badd = baddp.tile([128, 2, IB, L], BF16, tag="badd",
                          name=f"badd{hp}")
        nc.gpsimd.dma_start(
            badd[:],
            bgout[hp][:].rearrange("(b p) h c j -> p h b (c j)", p=128))
        nc.vector.tensor_tensor(zflat, zflat,
                                badd[:].rearrange("p h b j -> p (h b j)"),
                                ALU.add)
        nc.vector.tensor_reduce(nmx[:, hp, :, :], zslc, AX.X, ALU.max,
                                negate=True)
        for hh in range(2):
            h = 2 * hp + hh
            for ib in range(IB):
                nc.scalar.activation(asb[:, h, ib, :], zsb[:, h, ib, :],
                                     AF.Exp,
                                     bias=nmx[:, hp, hh, ib:ib + 1],
                                     accum_out=sums[:, hp, hh, ib:ib + 1])
        nc.vector.reciprocal(rec[:, hp, :, :], sums[:, hp, :, :])
        for hh in range(2):
            h = 2 * hp + hh
            for ib in range(IB):
                nc.vector.tensor_scalar_mul(asb[:, h, ib, :],
                                            asb[:, h, ib, :],
                                            rec[:, hp, hh, ib:ib + 1])
        # A^T via PE transposes, batched 6/3 per PSUM bank: at[j%128,h,jb,i]
        for hh in range(2):
            h = 2 * hp + hh
            pt = ps_tr.tile([128, 6, 128], BF16, tag="tr", name=f"at6_{h}")
            for idx in range(6):                 # ib 0..1 x jb 0..2
                ib, jb = divmod(idx, IB)
                nc.tensor.transpose(
                    pt[:, idx, :],
                    asb[:, h, ib, jb * 128:(jb + 1) * 128], ident[:])
            nc.vector.tensor_copy(
                at[:, h, :, 0:256].rearrange("p jb (ib q) -> p ib jb q",
                                             q=128),
                pt[:].rearrange("p (ib jb) q -> p ib jb q", jb=IB))
            pt2 = ps_tr.tile([128, 6, 128], BF16, tag="tr", name=f"at3_{h}")
            for jb in range(IB):                 # ib 2
                nc.tensor.transpose(
                    pt2[:, jb, :],
                    asb[:, h, 2, jb * 128:(jb + 1) * 128], ident[:])
            nc.scalar.copy(at[:, h, :, 256:384], pt2[:, 0:IB, :])
        # einsum A@V for this head pair; evict full width to stg, then
        # DVE-4x partition shifts into the gated layout
        for gq in range(NG // 2):
            for hh in range(2):
                h = 2 * hp + hh
                ep = ps_acc.tile([128, 2, 512], F32, tag="acc")
                for gg in range(2):
                    gidx = 2 * gq + gg
                    for jb in range(IB):
                        nc.tensor.matmul(
                            ep[:, gg, 0:L],
                            vt[:, jb, h, gidx, :, :].rearrange(
                                "p a b -> p (a b)"),
                            at[:, h, jb, :], start=(jb == 0),
                            stop=(jb == IB - 1))
                stg = stgp.tile([128, 2, L], BF16, tag="stg")
                if hh % 2 == 0:
                    nc.scalar.copy(stg[:], ep[:, :, 0:L])
                else:
                    nc.vector.tensor_copy(stg[:], ep[:, :, 0:L])
                # partition shifts via HWDGE SBUF DMAs (keeps DVE free),
                # alternating between the SP and ACT queues
                for nn in range(4):
                    dst = gated_r[32 * h:32 * h + 32, gq, :, nn, :]
                    eng = nc.sync if nn % 2 == 0 else nc.scalar
                    eng.dma_start(dst, stg[32 * nn:32 * nn + 32, :, :])
            # after the second head pair completes a gq, gate + project + out
            if hp == 1:
                nc.vector.tensor_tensor(
                    gated[:, 8 * gq:8 * gq + 8, :],
                    gated[:, 8 * gq:8 * gq + 8, :],
                    gsb[:, 8 * gq:8 * gq + 8, :], ALU.mult)
                for fb in range(6):
                    fp = ps_tr.tile([128, 4, 128], F32, tag="tr")
                    for q in range(4):
                        rb = gq * 24 + fb * 4 + q
                        nc.tensor.matmul(fp[:, q, :],
                                         gated_flat[:, rb * 128:(rb + 1) * 128],
                                         wot_b[:], start=True, stop=True)
                    fst = outstg.tile([128, 4, 128], F32, tag="fst")
                    if fb % 2 == 0:
                        nc.scalar.copy(fst[:], fp[:])
                    else:
                        nc.vector.tensor_copy(fst[:], fp[:])
                    fbg = gq * 6 + fb
                    nc.sync.dma_start(outc[:, fbg * 4:(fbg + 1) * 4, :],
                                      fst[:])


# ---------------------------------------------------------------------------
_NC_CACHE = {}


def _build_program():
    if "nc" in _NC_CACHE:
        return _NC_CACHE["nc"]
    nc = bacc.Bacc("TRN2", target_bir_lowering=False, debug=False,
                   enable_asserts=False, num_devices=NCORES)
    ins = {
        "pairc": nc.dram_tensor("pairc", [R, DP], F32, kind="ExternalInput").ap(),
        "biasc": nc.dram_tensor("biasc", [R, DP], F32, kind="ExternalInput").ap(),
        "wqt": nc.dram_tensor("wqt", [DP, HD], F32, kind="ExternalInput").ap(),
        "wkt": nc.dram_tensor("wkt", [DP, HD], F32, kind="ExternalInput").ap(),
        "wvt": nc.dram_tensor("wvt", [DP, HD], F32, kind="ExternalInput").ap(),
        "wgt": nc.dram_tensor("wgt", [DP, HD], F32, kind="ExternalInput").ap(),
        "wot": nc.dram_tensor("wot", [HD, DP], F32, kind="ExternalInput").ap(),
        "wbt": nc.dram_tensor("wbt", [DP, H], F32, kind="ExternalInput").ap(),
        "vecs": nc.dram_tensor("vecs", [DP, 4], F32, kind="ExternalInput").ap(),
    }
    outs = {
        "outc": nc.dram_tensor("outc", [R, DP], F32, kind="ExternalOutput").ap(),
    }
    with tile.TileContext(nc) as tc:
        with ExitStack() as ctx:
            build_tile_kernel(ctx, tc, outs, ins)
    nc.compile()
    _NC_CACHE["nc"] = nc
    return nc


def shard_inputs(pair, bias, ln_pair_g, ln_pair_b, ln_bias_g, ln_bias_b,
                 Wq, Wk, Wv, Wb, Wg, bg, Wo, bo):
    """Host-side slicing/permutation -> per-core input maps."""
    assert pair.shape == (O, L, L, DP) and bias.shape == (O, L, L, DP)
    assert np.abs(ln_pair_b).max() == 0 and np.abs(ln_bias_b).max() == 0, \
        "kernel folds LN beta=0; nonzero beta not implemented"
    f32 = np.float32
    shared = {
        "wqt": np.ascontiguousarray(Wq.T, f32),
        "wkt": np.ascontiguousarray(Wk.T, f32),
        "wvt": np.ascontiguousarray(Wv.T, f32),
        "wgt": np.ascontiguousarray(Wg.T, f32),
        "wot": np.ascontiguousarray(Wo.T, f32),
        "wbt": np.ascontiguousarray(Wb.T, f32),
        "vecs": np.ascontiguousarray(
            np.stack([ln_pair_g, ln_bias_g, bg, bo], axis=1), f32),
    }
    in_maps = []
    for c in range(NCORES):
        S = slice(c * NS, (c + 1) * NS)
        m = dict(shared)
        m["pairc"] = np.ascontiguousarray(
            pair[0][:, S, :].transpose(1, 0, 2).reshape(R, DP), f32)
        m["biasc"] = np.ascontiguousarray(
            bias[0][:, S, :].transpose(1, 0, 2).reshape(R, DP), f32)
        in_maps.append(m)
    return in_maps


def gather_outputs(results, bo):
    res = np.zeros((O, L, L, DP), np.float32)
    for c in range(NCORES):
        F = results[c]["outc"].reshape(NS, L, DP)
        res[0, :, c * NS:(c + 1) * NS, :] = F.transpose(1, 0, 2)
    res += np.asarray(bo, np.float32)          # bo folded on the host
    return res


def kernel(**inputs):
    inputs = {k: np.asarray(v) for k, v in inputs.items()}
    nc = _build_program()
    in_maps = shard_inputs(**inputs)
    r = run_bass_kernel_spmd(nc, in_maps, core_ids=list(range(NCORES)))
    return gather_outputs(r.results, inputs["bo"])


def _ensure_ntff_hook():
    """The agent image's antenv lacks axon_hooks; recreate the registry and
    wire the ctypes NTFF hook from trn_agent_boot (profiling-only path)."""
    try:
        from antenv.axon_hooks import get_axon_ntff_profile_hook  # noqa: F401
        return
    except ImportError:
        pass
    import types
    import antenv
    mod = types.ModuleType("antenv.axon_hooks")
    mod._hook = None
    mod.set_axon_ntff_profile_hook = lambda h: setattr(mod, "_hook", h)
    mod.get_axon_ntff_profile_hook = lambda: mod._hook
    sys.modules["antenv.axon_hooks"] = mod
    antenv.axon_hooks = mod
    try:
        from trn_agent_boot.trn_boot import _ntff_profile_via_ctypes
        hook = _ntff_profile_via_ctypes("/opt/axon/libaxon_pjrt.so")
        if hook is not None:
            mod._hook = hook
    except Exception as e:  # profiling degrades, run still works
        print(f"NTFF hook setup failed: {e}", file=sys.stderr)


def kernel_profiled(**inputs):
    """Like kernel() but also returns exec-time info from neuron-profile."""
    inputs = {k: np.asarray(v) for k, v in inputs.items()}
    _ensure_ntff_hook()
    import concourse.bass_utils as bu
    bu.upload_artifacts = lambda tmpdir: f"local:{tmpdir}"  # no bucket here
    nc = _build_program()
    in_maps = shard_inputs(**inputs)
    r = run_bass_kernel_spmd(nc, in_maps, core_ids=list(range(NCORES)),
                             trace=True, trace_cores=list(range(NCORES)))
    return gather_outputs(r.results, inputs["bo"]), r


# revision 30
# speedup vs baseline: 1.1213x; 1.1213x over previous
"""Biased axial attention (RoseTTAFold-style) on 8 TRN2 NeuronCores.

nn_BiasedAxialAttention: O=1, L=384, d_pair=d_bias=128, H=4, DH=32.

  p    = LN(pair^T);  bsrc = LN(bias^T)            (LN over d per position)
  q,k,v,gate projections of p; b = bsrc @ Wb^T
  attn[i,j,h] = sum_{n,k} q[n,i,h,k] k[n,j,h,k] + b[i,j,h]
  A = softmax_j(attn);  out[n,i,:] = (gate * einsum(A, v)) @ Wo^T + bo
  result[i,n,:] = out[n,i,:]

Sharding: the non-attended axial dim n, 48 rows/core. Each core computes
partial logits for its n-slice; two on-chip AllReduces (heads 01 / 23) sum
them. The bias term B (each core computes its i-slice) is shared via two
AllGathers. bo is added on the host during the gather.

v2 pipeline highlights (vs the v1 baseline):
  - LN: batched bn_stats ([128,4,128] per op), fused stats arithmetic on
    GpSimd, DVE-4x/ACT normalize, and the feature transpose via the DMA
    XBAR (dma_start transpose=True) straight into X^T -- no PE transpose,
    no PSUM eviction.
  - Q/K projections packed two-per-PSUM-bank-pair, wide evictions, and the
    (h,k)->(k,nn) partition restack batched as one DMA per (head, 2 groups).
  - logits h-split into two AllReduces so softmax/einsum pipeline by
    head-pair; B projection runs j-major so its evictions are [128,16]
    wide instead of [4,384].
  - softmax: one batched max-reduce per head-pair, exp with fused
    accumulation, A^T via DMA XBAR transpose.
  - einsum: full-width [128,2,384] PSUM evictions + DVE-4x partition
    shifts into the gated layout; gate multiply on GpSimd; output
    projection PSUM-batched 4 row-blocks per bank.
"""
import sys

if "/opt/trn_rl_repo" not in sys.path:
    sys.path.insert(0, "/opt/trn_rl_repo")

import numpy as np
from contextlib import ExitStack

import concourse.bass as bass
import concourse.bacc as bacc
import concourse.mybir as mybir
import concourse.tile as tile
from concourse.bass_utils import run_bass_kernel_spmd
from concourse.masks import make_identity

F32 = mybir.dt.float32
BF16 = mybir.dt.bfloat16
AF = mybir.ActivationFunctionType
ALU = mybir.AluOpType
AX = mybir.AxisListType

O, L, DP, H, DH = 1, 384, 128, 4, 32
HD = H * DH
NCORES = 8
NS = L // NCORES            # 48 n's per core
R = NS * L                  # 18432 rows per core
NBLK = R // 128             # 144 row-blocks
NG = NS // 4                # 12 logit contraction groups (4 n's each)
IB = L // 128               # 3 blocks of 128 along i/j
SCALING = 1.0 / np.sqrt(DH)
EPS = 1e-5

CH = 12                     # LN chunk: 12 row-blocks = 1536 rows = one group
NCHUNK = NBLK // CH         # 12 chunks per tensor


def build_tile_kernel(ctx: ExitStack, tc: tile.TileContext, outs, ins):
    nc = tc.nc
    pairc = ins["pairc"].rearrange("(b p) d -> p b d", p=128)   # [128,144,128]
    biasc = ins["biasc"].rearrange("(b p) d -> p b d", p=128)
    outc = outs["outc"].rearrange("(b p) d -> p b d", p=128)

    const = ctx.enter_context(tc.tile_pool(name="const", bufs=1))
    big = ctx.enter_context(tc.tile_pool(name="big", bufs=1))
    lnload = ctx.enter_context(tc.tile_pool(name="lnload", bufs=3))
    stream = ctx.enter_context(tc.tile_pool(name="stream", bufs=2))
    bstream = ctx.enter_context(tc.tile_pool(name="bstream", bufs=3))
    evqp = ctx.enter_context(tc.tile_pool(name="evqp", bufs=2))
    stgp = ctx.enter_context(tc.tile_pool(name="stgp", bufs=3))
    baddp = ctx.enter_context(tc.tile_pool(name="baddp", bufs=2))
    outstg = ctx.enter_context(tc.tile_pool(name="outstg", bufs=2))
    ps_acc = ctx.enter_context(tc.tile_pool(name="psacc", bufs=2, space="PSUM"))
    ps_tr = ctx.enter_context(tc.tile_pool(name="pstr", bufs=2, space="PSUM"))
    ps_pj = ctx.enter_context(tc.tile_pool(name="pspj", bufs=1, space="PSUM"))
    dram = ctx.enter_context(tc.tile_pool(name="dram", bufs=1, space="DRAM"))

    # ---------------- stage 0: constants / weights -------------------------
    ident = const.tile([128, 128], BF16)
    make_identity(nc, ident)
    epst = const.tile([128, 1], F32)
    nc.vector.memset(epst[:], EPS)

    vecs = const.tile([128, 4], F32)          # cols: gp, gb, bg, bo(unused)
    nc.sync.dma_start(vecs[:], ins["vecs"][:])

    wf = {}
    for nm in ("wqt", "wkt", "wvt", "wgt", "wot"):
        t = const.tile([128, 128], F32, name=f"{nm}_f")
        nc.sync.dma_start(t[:], ins[nm][:])
        wf[nm] = t
    wbt_f = const.tile([128, 4], F32)
    nc.sync.dma_start(wbt_f[:], ins["wbt"][:])

    # fold LN gain + scale constants into bf16 lhsT weights (per-partition d)
    wb = {}
    for nm, extra in (("wqt", SCALING), ("wkt", 1.0 / L), ("wvt", 1.0),
                      ("wgt", 1.0)):
        gs = const.tile([128, 1], F32, name=f"{nm}_gs")
        nc.vector.tensor_scalar_mul(gs[:], vecs[:, 0:1], extra)
        t = const.tile([128, 128], BF16, name=f"{nm}_b")
        nc.vector.tensor_scalar_mul(t[:], wf[nm][:], gs[:, 0:1])
        wb[nm] = t
    wot_b = const.tile([128, 128], BF16)
    nc.vector.tensor_copy(wot_b[:], wf["wot"][:])
    wbt_b = const.tile([128, 4], BF16)
    nc.vector.tensor_scalar_mul(wbt_b[:], wbt_f[:], vecs[:, 1:2])

    # ---------------- persistent SBUF / DRAM tensors -----------------------
    xt = big.tile([128, NS, L], BF16, tag="xt")        # X^T [d,(n,i)]
    qs = big.tile([128, H, NG, L], BF16, tag="qs")     # [(k,nn), h, g, i]
    ks = big.tile([128, H, NG, L], BF16, tag="ks")
    zsb = big.tile([128, H, IB, L], BF16, tag="z")     # logits [i%128,h,ib,j]
    asb = big.tile([128, H, IB, L], BF16, tag="a")     # softmax(A)
    at = big.tile([128, H, IB, L], BF16, tag="at")     # A^T [j%128, h, jb, i]

    zin, zout, bgin, bgout = [], [], [], []
    for hp in range(2):
        zin.append(dram.tile([128, 2 * IB * L], BF16, name=f"zin{hp}"))
        zout.append(dram.tile([128, 2 * IB * L], BF16, addr_space="Shared",
                              name=f"zout{hp}"))
        bgin.append(dram.tile([NS, 2, IB, 128], BF16, name=f"bgin{hp}"))
        bgout.append(dram.tile([L, 2, IB, 128], BF16, addr_space="Shared",
                               name=f"bgout{hp}"))

    # ---------------- LN chunk: stats + normalize + XBAR transpose ---------
    # bn_stats is batched 4 blocks/op; the stats arithmetic is fused into 6
    # GpSimd ops on [128,12] tiles; r = rsqrt(var+eps) in one ACT op.
    # The feature-major transpose goes through the DMA XBAR straight into
    # the destination -- no PE transpose, no PSUM round-trip.
    def ln_chunk(src_dram, ch, dst_ap):
        rm = lnload.tile([128, CH, 128], BF16, tag="lnrm")
        nc.gpsimd.dma_start(rm[:], src_dram[:, ch * CH:(ch + 1) * CH, :])
        st = stream.tile([128, CH, 6], F32, tag="lnst")
        for b in range(CH):
            nc.vector.bn_stats(st[:, b, :], rm[:, b, :])
        dm = stream.tile([128, CH], F32, tag="lndm")
        v0 = stream.tile([128, CH], F32, tag="lnv0")
        r = stream.tile([128, CH], F32, tag="lnr")
        nmr = stream.tile([128, CH], F32, tag="lnnmr")
        g = nc.gpsimd
        # var = (cv_e+cv_o)/128 + ((m_e-m_o)/2)^2 ; mean = (m_e+m_o)/2
        # (Pool only runs plain tensor_tensor; the fused scalar ops are DVE)
        g.tensor_tensor(dm[:], st[:, :, 1], st[:, :, 4], ALU.subtract)
        nc.vector.scalar_tensor_tensor(dm[:], dm[:], 0.25, dm[:], ALU.mult,
                                       ALU.mult)
        g.tensor_tensor(v0[:], st[:, :, 2], st[:, :, 5], ALU.add)
        nc.vector.scalar_tensor_tensor(v0[:], v0[:], 1.0 / 128, dm[:],
                                       ALU.mult, ALU.add)
        nc.scalar.activation(r[:], v0[:], AF.Sqrt, bias=epst[:, 0:1])
        nc.vector.reciprocal(r[:], r[:])
        g.tensor_tensor(nmr[:], st[:, :, 1], st[:, :, 4], ALU.add)
        nc.vector.scalar_tensor_tensor(nmr[:], nmr[:], -0.5, r[:], ALU.mult,
                                       ALU.mult)
        norm = stream.tile([128, CH, 128], BF16, tag="lnnorm")
        for b in range(CH):
            if b % 2 == 1:
                nc.scalar.activation(norm[:, b, :], rm[:, b, :], AF.Identity,
                                     bias=nmr[:, b:b + 1], scale=r[:, b:b + 1])
            else:
                nc.vector.tensor_scalar(norm[:, b, :], rm[:, b, :],
                                        r[:, b:b + 1], nmr[:, b:b + 1],
                                        ALU.mult, ALU.add)
        # PE transpose in 6-block halves, one batched 2x eviction per half
        for half in range(2):
            pt = ps_tr.tile([128, 6, 128], BF16, tag="tr")
            for b in range(6):
                nc.tensor.transpose(pt[:, b, :], norm[:, 6 * half + b, :],
                                    ident[:])
            if half % 2 == 0:
                nc.vector.tensor_copy(dst_ap[:, 6 * half:6 * half + 6, :],
                                      pt[:])
            else:
                nc.scalar.copy(dst_ap[:, 6 * half:6 * half + 6, :], pt[:])

    # ---------------- stage 1+3: pair LN + Q/K projections -----------------
    # qs/ks partition order within a group is p = 4k+nn; one restack DMA per
    # (head, 2 groups) turns evq [32k, gp, nn, i] into qs [(k nn), gp, i].
    # B projection runs j-major: out[j%128, il, h] tiles make the evictions
    # [128,16] instead of [4,384]; a tiny PE transpose flips each chunk to
    # row form [(h jb il), j%128] so the bgin DMA writes 256B runs.
    def bias_chunk(ch):
        btc = bstream.tile([128, 4, L], BF16, tag="btc")
        ln_chunk(biasc, ch,
                 btc[:].rearrange("p n (b q) -> p (n b) q", q=128))
        btj = bstream.tile([128, H, IB, 4], BF16, tag="btj")
        for jb in range(IB):
            bp = ps_pj.tile([128, 4, H], F32, tag="pj")
            for il in range(4):
                nc.tensor.matmul(bp[:, il, :],
                                 btc[:, il, jb * 128:(jb + 1) * 128],
                                 wbt_b[:], start=True, stop=True)
            if jb % 2 == 0:
                nc.vector.tensor_copy(btj[:, :, jb, :],
                                      bp[:].transpose([0, 2, 1]))
            else:
                nc.scalar.copy(btj[:, :, jb, :], bp[:].transpose([0, 2, 1]))
        pt = ps_pj.tile([48, 128], BF16, tag="pjt")
        nc.tensor.transpose(pt[:],
                            btj[:].rearrange("p h b il -> p (h b il)"),
                            ident[:])
        bsg = bstream.tile([48, 128], BF16, tag="bsg")
        nc.vector.tensor_copy(bsg[:], pt[:])
        for hp in range(2):
            dst = bgin[hp][4 * ch:4 * ch + 4, :, :, :]
            nc.gpsimd.dma_start(dst.transpose([1, 2, 0, 3]),
                                bsg[24 * hp:24 * hp + 24, :])

    for g in range(NG):
        dst = xt[:, 4 * g:4 * g + 4, :].rearrange("p n (b q) -> p (n b) q",
                                                  q=128)
        ln_chunk(pairc, g, dst)
        evq = {}
        for wname in ("wqt", "wkt"):
            evq[wname] = evqp.tile([128, 4, L], BF16, tag=f"evq_{wname}",
                                   name=f"evq_{wname}_{g}")
        for wi, wname in enumerate(("wqt", "wkt")):
            for half in range(2):
                pp = ps_acc.tile([128, 2, 512], F32, tag="acc")
                for s in range(2):
                    nn = 2 * half + s
                    nc.tensor.matmul(pp[:, s, 0:L], wb[wname][:],
                                     xt[:, 4 * g + nn, :], start=True,
                                     stop=True)
                dst = evq[wname][:, 2 * half:2 * half + 2, :]
                if (wi + half) % 2 == 0:
                    nc.vector.tensor_copy(dst, pp[:, :, 0:L])
                else:
                    nc.scalar.copy(dst, pp[:, :, 0:L])
        for h in range(H):
            for wname, dstb in (("wqt", qs), ("wkt", ks)):
                nc.sync.dma_start(dstb[:, h, g, :],
                                  evq[wname][32 * h:32 * h + 32, :, :])
        bias_chunk(g)

    # AllGathers issued before the AllReduces so the ring starts as soon as
    # the bias side lands; softmax h01 then only waits on AG01+AR1.
    for hp in range(2):
        nc.gpsimd.collective_compute(
            "AllGather", ALU.bypass, replica_groups=[list(range(NCORES))],
            ins=[bgin[hp][:].opt()], outs=[bgout[hp][:].opt()])

    # ---------------- stage 4: logits + h-split AllReduce ------------------
    # bf16 AllReduce carries only the tiny q.k partial sums; the dominant
    # bias term is added post-AR from the AllGather.
    for hp in range(2):
        for h in (2 * hp, 2 * hp + 1):
            for ib in range(IB):
                lp = ps_tr.tile([128, 512], F32, tag="tr")
                for gg in range(NG):
                    nc.tensor.matmul(lp[:, 0:L],
                                     qs[:, h, gg, ib * 128:(ib + 1) * 128],
                                     ks[:, h, gg, :], start=(gg == 0),
                                     stop=(gg == NG - 1))
                if (h + ib) % 2 == 0:
                    nc.vector.tensor_copy(zsb[:, h, ib, :], lp[:, 0:L])
                else:
                    nc.scalar.copy(zsb[:, h, ib, :], lp[:, 0:L])
        nc.sync.dma_start(
            zin[hp][:],
            zsb[:, 2 * hp:2 * hp + 2, :, :].rearrange("p h b j -> p (h b j)"))
        nc.gpsimd.collective_compute(
            "AllReduce", ALU.add, replica_groups=[list(range(NCORES))],
            ins=[zin[hp][:].opt()], outs=[zout[hp][:].opt()])

    # ---------------- stage 6: G + V projections (overlap AR) --------------
    gsb = big.tile([128, NS, L], BF16, tag="ks")       # reuses ks slot
    for np_ in range(NS // 2):
        gp_ = ps_acc.tile([128, 2, 512], F32, tag="acc")
        for s in range(2):
            nc.tensor.matmul(gp_[:, s, 0:L], wb["wgt"][:],
                             xt[:, 2 * np_ + s, :], start=True, stop=True)
        nc.scalar.activation(gsb[:, 2 * np_:2 * np_ + 2, :], gp_[:, :, 0:L],
                             AF.Sigmoid, bias=vecs[:, 2:3])

    # vt layout: [j%128, jb, h, g, nn, d] so the einsum stationary slice is
    # one contiguous 128-wide free dim
    vt = big.tile([128, IB, H, NG, 4, DH], BF16, tag="qs")   # reuses qs slot
    for g in range(NG):
        for jb in range(IB):
            vp = ps_tr.tile([128, 4, 128], F32, tag="tr")
            for s in range(4):
                nc.tensor.matmul(vp[:, s, :],
                                 xt[:, 4 * g + s, jb * 128:(jb + 1) * 128],
                                 wb["wvt"][:], start=True, stop=True)
            vdst = vt[:, jb, :, g, :, :]
            vsrc = vp[:].rearrange("p s (h d) -> p h s d", h=H)
            if (g + jb) % 2 == 0:
                nc.vector.tensor_copy(vdst, vsrc)
            else:
                nc.scalar.copy(vdst, vsrc)

    # ---------------- stage 5..10: per head-pair tail ----------------------
    gated = big.tile([128, NS, L], BF16, tag="xt")     # reuses xt slot
    gated_flat = gated[:].rearrange("p n l -> p (n l)")
    gated_r = gated[:].rearrange("p (a b c) l -> p a b c l", b=2, c=4)
    sums = stream.tile([128, 2, 2, IB], F32, tag="smsum")
    nmx = stream.tile([128, 2, 2, IB], F32, tag="smmax")
    rec = stream.tile([128, 2, 2, IB], F32, tag="smrec")

    for hp in range(2):
        zslc = zsb[:, 2 * hp:2 * hp + 2, :, :]
        zflat = zslc.rearrange("p h b j -> p (h b j)")
        nc.gpsimd.dma_start(zflat, zout[hp][:])
        badd = baddp.tile([128, 2, IB, L], BF16, tag="badd",
                          name=f"badd{hp}")
        nc.gpsimd.dma_start(
            badd[:],
            bgout[hp][:].rearrange("(b p) h c j -> p h b (c j)", p=128))
        nc.vector.tensor_tensor(zflat, zflat,
                                badd[:].rearrange("p h b j -> p (h b j)"),
                                ALU.add)
        nc.vector.tensor_reduce(nmx[:, hp, :, :], zslc, AX.X, ALU.max,
                                negate=True)
        for hh in range(2):
            h = 2 * hp + hh
            for ib in range(IB):
                nc.scalar.activation(asb[:, h, ib, :], zsb[:, h, ib, :],
                                     AF.Exp,
                                     bias=nmx[:, hp, hh, ib:ib + 1],
                                     accum_out=sums[:, hp, hh, ib:ib + 1])
        nc.vector.reciprocal(rec[:, hp, :, :], sums[:, hp, :, :])
        for hh in range(2):
            h = 2 * hp + hh
            for ib in range(IB):
                nc.vector.tensor_scalar_mul(asb[:, h, ib, :],
                                            asb[:, h, ib, :],
                                            rec[:, hp, hh, ib:ib + 1])
        # A^T via PE transposes, batched 6/3 per PSUM bank: at[j%128,h,jb,i]
        for hh in range(2):
            h = 2 * hp + hh
            pt = ps_tr.tile([128, 6, 128], BF16, tag="tr", name=f"at6_{h}")
            for idx in range(6):                 # ib 0..1 x jb 0..2
                ib, jb = divmod(idx, IB)
                nc.tensor.transpose(
                    pt[:, idx, :],
                    asb[:, h, ib, jb * 128:(jb + 1) * 128], ident[:])
            nc.vector.tensor_copy(
                at[:, h, :, 0:256].rearrange("p jb (ib q) -> p ib jb q",
                                             q=128),
                pt[:].rearrange("p (ib jb) q -> p ib jb q", jb=IB))
            pt2 = ps_tr.tile([128, 6, 128], BF16, tag="tr", name=f"at3_{h}")
            for jb in range(IB):                 # ib 2
                nc.tensor.transpose(
                    pt2[:, jb, :],
                    asb[:, h, 2, jb * 128:(jb + 1) * 128], ident[:])
            nc.scalar.copy(at[:, h, :, 256:384], pt2[:, 0:IB, :])
        # einsum A@V for this head pair; evict full width to stg, then
        # DVE-4x partition shifts into the gated layout
        for gq in range(NG // 2):
            for hh in range(2):
                h = 2 * hp + hh
                ep = ps_acc.tile([128, 2, 512], F32, tag="acc")
                for gg in range(2):
                    gidx = 2 * gq + gg
                    for jb in range(IB):
                        nc.tensor.matmul(
                            ep[:, gg, 0:L],
                            vt[:, jb, h, gidx, :, :].rearrange(
                                "p a b -> p (a b)"),
                            at[:, h, jb, :], start=(jb == 0),
                            stop=(jb == IB - 1))
                stg = stgp.tile([128, 2, L], BF16, tag="stg")
                if hh % 2 == 0:
                    nc.scalar.copy(stg[:], ep[:, :, 0:L])
                else:
                    nc.vector.tensor_copy(stg[:], ep[:, :, 0:L])
                # partition shifts via sync-queue SBUF DMAs (keeps DVE free)
                for nn in range(4):
                    dst = gated_r[32 * h:32 * h + 32, gq, :, nn, :]
                    nc.sync.dma_start(dst, stg[32 * nn:32 * nn + 32, :, :])
            # after the second head pair completes a gq, gate + project + out
            if hp == 1:
                nc.gpsimd.tensor_tensor(
                    gated[:, 8 * gq:8 * gq + 8, :],
                    gated[:, 8 * gq:8 * gq + 8, :],
                    gsb[:, 8 * gq:8 * gq + 8, :], ALU.mult)
                for fb in range(6):
                    fp = ps_tr.tile([128, 4, 128], F32, tag="tr")
                    for q in range(4):
                        rb = gq * 24 + fb * 4 + q
                        nc.tensor.matmul(fp[:, q, :],
                                         gated_flat[:, rb * 128:(rb + 1) * 128],
                                         wot_b[:], start=True, stop=True)
                    fst = outstg.tile([128, 4, 128], F32, tag="fst")
                    if fb % 2 == 0:
                        nc.scalar.copy(fst[:], fp[:])
                    else:
                        nc.vector.tensor_copy(fst[:], fp[:])
                    fbg = gq * 6 + fb
                    nc.sync.dma_start(outc[:, fbg * 4:(fbg + 1) * 4, :],
                                      fst[:])


# ---------------------------------------------------------------------------
_NC_CACHE = {}


def _build_program():
    if "nc" in _NC_CACHE:
        return _NC_CACHE["nc"]
    nc = bacc.Bacc("TRN2", target_bir_lowering=False, debug=False,
                   enable_asserts=False, num_devices=NCORES)
    ins = {
        "pairc": nc.dram_tensor("pairc", [R, DP], F32, kind="ExternalInput").ap(),
        "biasc": nc.dram_tensor("biasc", [R, DP], F32, kind="ExternalInput").ap(),
        "wqt": nc.dram_tensor("wqt", [DP, HD], F32, kind="ExternalInput").ap(),
        "wkt": nc.dram_tensor("wkt", [DP, HD], F32, kind="ExternalInput").ap(),
        "wvt": nc.dram_tensor("wvt", [DP, HD], F32, kind="ExternalInput").ap(),
        "wgt": nc.dram_tensor("wgt", [DP, HD], F32, kind="ExternalInput").ap(),
        "wot": nc.dram_tensor("wot", [HD, DP], F32, kind="ExternalInput").ap(),
        "wbt": nc.dram_tensor("wbt", [DP, H], F32, kind="ExternalInput").ap(),
        "vecs": nc.dram_tensor("vecs", [DP, 4], F32, kind="ExternalInput").ap(),
    }
    outs = {
        "outc": nc.dram_tensor("outc", [R, DP], F32, kind="ExternalOutput").ap(),
    }
    with tile.TileContext(nc) as tc:
        with ExitStack() as ctx:
            build_tile_kernel(ctx, tc, outs, ins)
    nc.compile()
    _NC_CACHE["nc"] = nc
    return nc


def shard_inputs(pair, bias, ln_pair_g, ln_pair_b, ln_bias_g, ln_bias_b,
                 Wq, Wk, Wv, Wb, Wg, bg, Wo, bo):
    """Host-side slicing/permutation -> per-core input maps."""
    assert pair.shape == (O, L, L, DP) and bias.shape == (O, L, L, DP)
    assert np.abs(ln_pair_b).max() == 0 and np.abs(ln_bias_b).max() == 0, \
        "kernel folds LN beta=0; nonzero beta not implemented"
    f32 = np.float32
    shared = {
        "wqt": np.ascontiguousarray(Wq.T, f32),
        "wkt": np.ascontiguousarray(Wk.T, f32),
        "wvt": np.ascontiguousarray(Wv.T, f32),
        "wgt": np.ascontiguousarray(Wg.T, f32),
        "wot": np.ascontiguousarray(Wo.T, f32),
        "wbt": np.ascontiguousarray(Wb.T, f32),
        "vecs": np.ascontiguousarray(
            np.stack([ln_pair_g, ln_bias_g, bg, bo], axis=1), f32),
    }
    in_maps = []
    for c in range(NCORES):
        S = slice(c * NS, (c + 1) * NS)
        m = dict(shared)
        m["pairc"] = np.ascontiguousarray(
            pair[0][:, S, :].transpose(1, 0, 2).reshape(R, DP), f32)
        m["biasc"] = np.ascontiguousarray(
            bias[0][:, S, :].transpose(1, 0, 2).reshape(R, DP), f32)
        in_maps.append(m)
    return in_maps


def gather_outputs(results, bo):
    res = np.zeros((O, L, L, DP), np.float32)
    for c in range(NCORES):
        F = results[c]["outc"].reshape(NS, L, DP)
        res[0, :, c * NS:(c + 1) * NS, :] = F.transpose(1, 0, 2)
    res += np.asarray(bo, np.float32)          # bo folded on the host
    return res


def kernel(**inputs):
    inputs = {k: np.asarray(v) for k, v in inputs.items()}
    nc = _build_program()
    in_maps = shard_inputs(**inputs)
    r = run_bass_kernel_spmd(nc, in_maps, core_ids=list(range(NCORES)))
    return gather_outputs(r.results, inputs["bo"])


def _ensure_ntff_hook():
    """The agent image's antenv lacks axon_hooks; recreate the registry and
    wire the ctypes NTFF hook from trn_agent_boot (profiling-only path)."""
    try:
        from antenv.axon_hooks import get_axon_ntff_profile_hook  # noqa: F401
        return
    except ImportError:
        pass
    import types
    import antenv
    mod = types.ModuleType("antenv.axon_hooks")
    mod._hook = None
    mod.set_axon_ntff_profile_hook = lambda h: setattr(mod, "_hook", h)
    mod.get_axon_ntff_profile_hook = lambda: mod._hook
    sys.modules["antenv.axon_hooks"] = mod
    antenv.axon_hooks = mod
    try:
        from trn_agent_boot.trn_boot import _ntff_profile_via_ctypes
        hook = _ntff_profile_via_ctypes("/opt/axon/libaxon_pjrt.so")
        if hook is not None:
            mod._hook = hook
    except Exception as e:  # profiling degrades, run still works
        print(f"NTFF hook setup failed: {e}", file=sys.stderr)


def kernel_profiled(**inputs):
    """Like kernel() but also returns exec-time info from neuron-profile."""
    inputs = {k: np.asarray(v) for k, v in inputs.items()}
    _ensure_ntff_hook()
    import concourse.bass_utils as bu
    bu.upload_artifacts = lambda tmpdir: f"local:{tmpdir}"  # no bucket here
    nc = _build_program()
    in_maps = shard_inputs(**inputs)
    r = run_bass_kernel_spmd(nc, in_maps, core_ids=list(range(NCORES)),
                             trace=True, trace_cores=list(range(NCORES)))
    return gather_outputs(r.results, inputs["bo"]), r
